# Initial kernel scaffold
#
"""Trainium2 Bass kernel for the canonical Lp-ECE KDE calibration loss.

Reference computation (see problem statement):
    probs = softmax(input, axis=1)[:, :, ::8, ::8]       -> f [N=8192, C=19]
    y     = argmax(target, axis=1)[:, ::8, ::8]          -> [N]
    alphas = f/0.02 + 1
    log_kern[i,j] = log(f[i]) . (alphas[j]-1) - log_beta[j]   (diag = -inf)
    kern = exp(log_kern);  ratio = (kern @ onehot(y)) / rowsum(kern)
    loss = mean_i sum_c (ratio - f)^2

The O(N^2) part (two GEMMs + 67M exps) runs on 8 NeuronCores, row-sharded:
core k owns rows i in [k*1024, (k+1)*1024).  The j (kernel-center) axis is
*rotated* per core by k*1024 so the self-interaction diagonal always lands
at jlocal == ilocal in [0, 1024) -- a single compiled program (SPMD) can
then mask the diagonal at compile-time-known positions.

Device pipeline per core (ACT-engine bound: ~67M exps / 8 cores):
  GEMM1 (PE):  lognumT[j,i] = sum_K stat[K,j]*mov[K,i] - log_beta[j] in
               float32r (K=20), j-tile t on PE row-group t%4 so any 4
               consecutive tiles run 4-wide concurrently (tile_position).
  diag  (DVE): add -1e30 at diagonal positions of the 8 overlap tiles.
  exp   (ACT): kernT = exp(psum) -> bf16 sbuf; [128,1536] psum units
               (one ACT instruction per 3 j-tiles) -- the ACT engine is
               the bottleneck at ~67us/core (1 elem/lane/cycle @ 1.2GHz).
  GEMM2 (PE):  kern_yT[c,i] += y1[j,c]*kernT[j,i] accumulated in psum over
               all 64 j-tiles, col-group t%4 (3 concurrent per unit).
  epilogue:    combine col groups + den select (PE), clip + fast
               reciprocal, broadcast (PE), ratio, (ratio-f)^2, reduce ->
               per-core partial loss sum; host sums 8 partials / N.
"""

import numpy as np
import ml_dtypes
from scipy.special import gammaln

import concourse.bass as bass
import concourse.bacc as bacc
import concourse.tile as tile
from concourse import mybir
from concourse.bass_utils import run_bass_kernel_spmd

BF16 = mybir.dt.bfloat16
F32 = mybir.dt.float32
F32R = mybir.dt.float32r
NPBF16 = ml_dtypes.bfloat16

N = 8192          # total pixels after downsampling: 2*64*64
C = 19            # classes
C1 = C + 1        # classes + ones column (row-sum)
NCORES = 8
R = N // NCORES   # rows per core = 1024
K1 = 20           # f32r contraction rows: 19 classes + 1 row for -log_beta
NT = N // 128     # j tiles = 64
BW = np.float32(0.02)
DF = 8
BIGNEG = -1.0e30


def _build_nc():
    nc = bacc.Bacc(None, target_bir_lowering=False, debug=False)

    stat_d = nc.dram_tensor("stat", [128, N // 4], F32R, kind="ExternalInput")
    mov_d = nc.dram_tensor("mov", [128, R], F32R, kind="ExternalInput")
    y1_d = nc.dram_tensor("y1", [128, NT, C1], BF16, kind="ExternalInput")
    ftc_d = nc.dram_tensor("ftc", [C, R], F32, kind="ExternalInput")
    out_d = nc.dram_tensor("out", [1, 1], F32, kind="ExternalOutput")

    diag_np = (np.eye(128) * BIGNEG).astype(np.float32)
    comb_np = np.zeros((128, C1), dtype=NPBF16)
    for g in range(4):
        for c in range(C1):
            comb_np[32 * g + c, c] = 1.0
    dsel_np = np.zeros((128, 1), dtype=NPBF16)
    for g in range(4):
        dsel_np[32 * g + C, 0] = 1.0
    diag_d = nc.inline_tensor(diag_np, name="diagmask")
    comb_d = nc.inline_tensor(comb_np, name="combmat")
    dsel_d = nc.inline_tensor(dsel_np, name="denselect")

    from contextlib import ExitStack

    with tile.TileContext(nc) as tc, ExitStack() as ctx:
        consts = ctx.enter_context(tc.tile_pool(name="consts", bufs=1))
        kpool = ctx.enter_context(tc.tile_pool(name="kern", bufs=6))
        epool = ctx.enter_context(tc.tile_pool(name="epi", bufs=3))
        pln = ctx.enter_context(
            tc.tile_pool(name="pln", bufs=2, space=bass.MemorySpace.PSUM)
        )
        pacc = ctx.enter_context(
            tc.tile_pool(name="pacc", bufs=2, space=bass.MemorySpace.PSUM)
        )

        stat_sb = consts.tile([128, N // 4], F32R)
        nc.sync.dma_start(out=stat_sb[:, 0:128], in_=stat_d[:, 0:128])
        diag_sb = consts.tile([128, 128], F32)
        nc.sync.dma_start(out=diag_sb[:], in_=diag_d[:])
        for sl in (slice(128, 512), slice(512, 1024), slice(1024, 2048)):
            nc.sync.dma_start(out=stat_sb[:, sl], in_=stat_d[:, sl])
        mov_sb = consts.tile([128, R], F32R)
        nc.gpsimd.dma_start(out=mov_sb[:, 0:512], in_=mov_d[:, 0:512])
        y1_sb = consts.tile([128, NT, C1], BF16)
        nc.gpsimd.dma_start(out=y1_sb[:], in_=y1_d[:])
        nc.gpsimd.dma_start(out=mov_sb[:, 512:R], in_=mov_d[:, 512:R])
        ftc_sb = consts.tile([C, R], F32)
        nc.gpsimd.dma_start(out=ftc_sb[:], in_=ftc_d[:])
        comb_sb = consts.tile([128, C1], BF16)
        nc.gpsimd.dma_start(out=comb_sb[:], in_=comb_d[:])
        dsel_sb = consts.tile([128, 1], BF16)
        nc.gpsimd.dma_start(out=dsel_sb[:], in_=dsel_d[:])

        ones_1xC = consts.tile([1, C], BF16)
        nc.vector.memset(ones_1xC[:], 1.0)

        lsum = consts.tile([C, 2], F32)

        UNITS = [[0]] + [list(range(s, s + 3)) for s in range(1, NT, 3)]

        def emit_units(ic, psB, units):
            isl = slice(ic * 512, (ic + 1) * 512)
            for ts_list in units:
                    w = len(ts_list)
                    unit = pln.tile([128, 512 * w], F32, tag="unit")
                    for pos, t in enumerate(ts_list):
                        g = t % 4
                        tloc = t // 4
                        nc.tensor.matmul(
                            unit[:, pos * 512 : (pos + 1) * 512],
                            lhsT=stat_sb[
                                32 * g : 32 * g + K1, tloc * 128 : (tloc + 1) * 128
                            ],
                            rhs=mov_sb[32 * g : 32 * g + K1, isl],
                            start=True,
                            stop=True,
                            tile_position=(32 * g, 0),
                            skip_group_check=True,
                        )
                    for pos, t in enumerate(ts_list):
                        if 4 * ic <= t < 4 * ic + 4:
                            off = pos * 512 + t * 128 - ic * 512
                            nc.vector.tensor_add(
                                unit[:, off : off + 128],
                                unit[:, off : off + 128],
                                diag_sb[:],
                            )
                    ksb = kpool.tile([128, 512 * w], BF16, tag="ksb")
                    nc.scalar.activation(
                        ksb[:], unit[:], mybir.ActivationFunctionType.Exp
                    )
                    for pos, t in enumerate(ts_list):
                        g = t % 4
                        nc.tensor.matmul(
                            psB[32 * g : 32 * g + C1, :],
                            lhsT=y1_sb[:, t, :],
                            rhs=ksb[:, pos * 512 : (pos + 1) * 512],
                            start=(t < 4),
                            stop=(t >= NT - 4),
                            skip_group_check=True,
                            tile_position=(0, 32 * g),
                        )

        def emit_epilogue(ic, psB):
            isl = slice(ic * 512, (ic + 1) * 512)
            # epilogue for this i-chunk (all small tiles)

            ky4 = epool.tile([128, 512], BF16, tag="ky4")
            nc.vector.tensor_copy(ky4[0:116, :], psB[0:116, :])
            psc = pacc.tile([128, 512], F32, tag="psB")
            nc.tensor.matmul(
                psc[32:33, :],
                lhsT=dsel_sb[0:116, :],
                rhs=ky4[0:116, :],
                start=True,
                stop=True,
                skip_group_check=True,
                tile_position=(0, 32),
            )
            nc.tensor.matmul(
                psc[0:C1, :],
                lhsT=comb_sb[0:116, :],
                rhs=ky4[0:116, :],
                start=True,
                stop=True,
                skip_group_check=True,
                tile_position=(0, 0),
            )
            ky = epool.tile([C1, 512], F32, tag="ky")
            if ic == 0:
                nc.vector.tensor_copy(ky[:], psc[0:C1, :])
            else:
                nc.scalar.copy(ky[:], psc[0:C1, :])
            dmx = epool.tile([1, 512], F32, tag="dmx")
            nc.vector.tensor_scalar_max(dmx[:], psc[32:33, :], 1e-10)
            rcp = epool.tile([1, 512], F32, tag="rcp")
            nc.vector.reciprocal_approx_fast(out=rcp[:], in_=dmx[:])
            rcpb = epool.tile([1, 512], BF16, tag="rcpb")
            nc.vector.tensor_copy(rcpb[:], rcp[:])
            psr = pacc.tile([128, 512], F32, tag="psB")
            nc.tensor.matmul(
                psr[0:C, :],
                lhsT=ones_1xC[:],
                rhs=rcpb[:],
                start=True,
                stop=True,
            )
            ratio = epool.tile([C, 512], F32, tag="ratio")
            nc.vector.tensor_mul(ratio[:], ky[0:C, :], psr[0:C, :])
            dd = epool.tile([C, 512], F32, tag="dd")
            nc.vector.tensor_sub(dd[:], ratio[:], ftc_sb[:, isl])
            dd2 = epool.tile([C, 512], F32, tag="dd2")
            nc.vector.tensor_mul(dd2[:], dd[:], dd[:])
            nc.vector.reduce_sum(
                lsum[:, ic : ic + 1], dd2[:], axis=mybir.AxisListType.X
            )

        psB0 = pacc.tile([128, 512], F32, tag="psB")
        emit_units(0, psB0, UNITS)
        psB1 = pacc.tile([128, 512], F32, tag="psB")
        emit_units(1, psB1, UNITS[:7])
        emit_epilogue(0, psB0)
        emit_units(1, psB1, UNITS[7:])
        emit_epilogue(1, psB1)

        lsr = epool.tile([C, 1], F32, tag="lsr")
        nc.vector.reduce_sum(lsr[:], lsum[:], axis=mybir.AxisListType.X)
        part = epool.tile([1, 1], F32, tag="part")
        nc.gpsimd.tensor_reduce(
            part[:], lsr[:], axis=mybir.AxisListType.XYZWC, op=mybir.AluOpType.add
        )
        nc.sync.dma_start(out=out_d[:], in_=part[:])

    nc.compile()
    return nc


_NC_CACHE = None


def _get_nc():
    global _NC_CACHE
    if _NC_CACHE is None:
        _NC_CACHE = _build_nc()
    return _NC_CACHE


def prepare_in_maps(input, target):
    """Host-side preprocessing: softmax/argmax on the strided subsample,
    log-Beta normalizers, bf16 hi/lo operand splits, and the per-core
    j-rotated layouts."""
    x = np.asarray(input)[:, :, ::DF, ::DF].astype(np.float32)
    t = np.asarray(target)[:, :, ::DF, ::DF]

    m = x.max(axis=1, keepdims=True)
    e = np.exp(x - m)
    probs = e / e.sum(axis=1, keepdims=True)
    f = probs.transpose(0, 2, 3, 1).reshape(-1, C).astype(np.float32)
    y = t.argmax(axis=1).reshape(-1)

    alphas = f / BW + np.float32(1.0)
    b = alphas - np.float32(1.0)
    logf = np.log(f)
    lb = (
        gammaln(alphas.astype(np.float64)).sum(axis=1)
        - gammaln(alphas.sum(axis=1, dtype=np.float64))
    ).astype(np.float32)
    nlb = -lb

    in_maps = []
    for k in range(NCORES):
        perm = (np.arange(N) + k * R) % N
        b_rot = b[perm]
        nlb_rot = nlb[perm]
        # j-tile t lives in row-group t%4 (partitions 32g..32g+19), packed
        # at column block t//4 -- no duplication needed for 4-wide matmuls
        stat = np.zeros((128, N // 4), dtype=np.float32)
        for g in range(4):
            sel = np.arange(NT // 4) * 4 + g          # tiles in this group
            cols = (sel[:, None] * 128 + np.arange(128)[None, :]).ravel()
            stat[32 * g : 32 * g + 19] = b_rot[cols].T
            stat[32 * g + 19] = nlb_rot[cols]

        rows = slice(k * R, (k + 1) * R)
        mov = np.zeros((128, R), dtype=np.float32)
        for g in range(4):
            mov[32 * g : 32 * g + 19] = logf[rows].T
            mov[32 * g + 19] = 1.0

        yp = y[perm].reshape(NT, 128)  # [t, p]
        y1 = np.zeros((128, NT, C1), dtype=NPBF16)
        onehot = (yp[:, :, None] == np.arange(C)[None, None, :]).astype(NPBF16)
        y1[:, :, 0:C] = onehot.transpose(1, 0, 2)
        y1[:, :, C] = NPBF16(1.0)

        ftc = np.ascontiguousarray(f[rows].T)

        in_maps.append({"stat": stat, "mov": mov, "y1": y1, "ftc": ftc})
    return in_maps


def run_device(in_maps, trace=False, trace_cores=None):
    nc = _get_nc()
    return run_bass_kernel_spmd(
        nc,
        in_maps,
        core_ids=list(range(NCORES)),
        trace=trace,
        trace_cores=trace_cores,
    )


def kernel(input, target):
    in_maps = prepare_in_maps(input, target)
    res = run_device(in_maps)
    total = np.float32(0.0)
    for r in res.results:
        total += r["out"][0, 0]
    return np.array(total / np.float32(N), dtype=np.float32)



# revision 1
# speedup vs baseline: 1.1375x; 1.1375x over previous
"""Trainium2 Bass kernel for the canonical Lp-ECE KDE calibration loss.

Reference computation (see problem statement):
    probs = softmax(input, axis=1)[:, :, ::8, ::8]       -> f [N=8192, C=19]
    y     = argmax(target, axis=1)[:, ::8, ::8]          -> [N]
    alphas = f/0.02 + 1
    log_kern[i,j] = log(f[i]) . (alphas[j]-1) - log_beta[j]   (diag = -inf)
    kern = exp(log_kern);  ratio = (kern @ onehot(y)) / rowsum(kern)
    loss = mean_i sum_c (ratio - f)^2

The O(N^2) part (two GEMMs + 67M exps) runs on 8 NeuronCores, row-sharded:
core k owns rows i in [k*1024, (k+1)*1024).  The j (kernel-center) axis is
*rotated* per core by k*1024 so the self-interaction diagonal always lands
at jlocal == ilocal in [0, 1024) -- a single compiled program (SPMD) can
then mask the diagonal at compile-time-known positions.

Device pipeline per core (ACT-engine bound: ~67M exps / 8 cores):
  GEMM1 (PE):  lognumT[j,i] = sum_K stat[K,j]*mov[K,i] - log_beta[j] in
               float32r (K=20), j-tile t on PE row-group t%4 so any 4
               consecutive tiles run 4-wide concurrently (tile_position).
  diag  (DVE): add -1e30 at diagonal positions of the 8 overlap tiles.
  exp   (ACT): kernT = exp(psum) -> bf16 sbuf; [128,1536] psum units
               (one ACT instruction per 3 j-tiles) -- the ACT engine is
               the bottleneck at ~67us/core (1 elem/lane/cycle @ 1.2GHz).
  GEMM2 (PE):  kern_yT[c,i] += y1[j,c]*kernT[j,i] accumulated in psum over
               all 64 j-tiles, col-group t%4 (3 concurrent per unit).
  epilogue:    combine col groups + den select (PE), clip + fast
               reciprocal, broadcast (PE), ratio, (ratio-f)^2, reduce ->
               per-core partial loss sum; host sums 8 partials / N.
"""

import numpy as np
import ml_dtypes
from scipy.special import gammaln

import concourse.bass as bass
import concourse.bacc as bacc
import concourse.tile as tile
from concourse import mybir
from concourse.bass_utils import run_bass_kernel_spmd

BF16 = mybir.dt.bfloat16
F32 = mybir.dt.float32
F32R = mybir.dt.float32r
NPBF16 = ml_dtypes.bfloat16

N = 8192          # total pixels after downsampling: 2*64*64
C = 19            # classes
C1 = C + 1        # classes + ones column (row-sum)
NCORES = 8
R = N // NCORES   # rows per core = 1024
K1 = 20           # f32r contraction rows: 19 classes + 1 row for -log_beta
NT = N // 128     # j tiles = 64
BW = np.float32(0.02)
DF = 8
BIGNEG = -1.0e30


def _build_nc():
    nc = bacc.Bacc(None, target_bir_lowering=False, debug=False)

    stat_d = nc.dram_tensor("stat", [128, N // 4], F32R, kind="ExternalInput")
    mov_d = nc.dram_tensor("mov", [128, R], F32R, kind="ExternalInput")
    y1_d = nc.dram_tensor("y1", [128, NT, C1], BF16, kind="ExternalInput")
    ftc_d = nc.dram_tensor("ftc", [C, R], F32, kind="ExternalInput")
    out_d = nc.dram_tensor("out", [1, 1], F32, kind="ExternalOutput")

    diag_np = (np.eye(128) * BIGNEG).astype(np.float32)
    comb_np = np.zeros((128, C1), dtype=NPBF16)
    for g in range(4):
        for c in range(C1):
            comb_np[32 * g + c, c] = 1.0
    dsel_np = np.zeros((128, 1), dtype=NPBF16)
    for g in range(4):
        dsel_np[32 * g + C, 0] = 1.0
    diag_d = nc.inline_tensor(diag_np, name="diagmask")
    comb_d = nc.inline_tensor(comb_np, name="combmat")
    dsel_d = nc.inline_tensor(dsel_np, name="denselect")

    from contextlib import ExitStack

    with tile.TileContext(nc) as tc, ExitStack() as ctx:
        consts = ctx.enter_context(tc.tile_pool(name="consts", bufs=1))
        kpool = ctx.enter_context(tc.tile_pool(name="kern", bufs=6))
        epool = ctx.enter_context(tc.tile_pool(name="epi", bufs=3))
        pln = ctx.enter_context(
            tc.tile_pool(name="pln", bufs=2, space=bass.MemorySpace.PSUM)
        )
        pacc = ctx.enter_context(
            tc.tile_pool(name="pacc", bufs=2, space=bass.MemorySpace.PSUM)
        )

        stat_sb = consts.tile([128, N // 4], F32R)
        nc.sync.dma_start(out=stat_sb[:, 0:128], in_=stat_d[:, 0:128])
        diag_sb = consts.tile([128, 128], F32)
        nc.sync.dma_start(out=diag_sb[:], in_=diag_d[:])
        for sl in (slice(128, 512), slice(512, 1024), slice(1024, 2048)):
            nc.sync.dma_start(out=stat_sb[:, sl], in_=stat_d[:, sl])
        mov_sb = consts.tile([128, R], F32R)
        nc.gpsimd.dma_start(out=mov_sb[:, 0:512], in_=mov_d[:, 0:512])
        y1_sb = consts.tile([128, NT, C1], BF16)
        nc.gpsimd.dma_start(out=y1_sb[:], in_=y1_d[:])
        nc.gpsimd.dma_start(out=mov_sb[:, 512:R], in_=mov_d[:, 512:R])
        ftc_sb = consts.tile([C, R], F32)
        nc.gpsimd.dma_start(out=ftc_sb[:], in_=ftc_d[:])
        comb_sb = consts.tile([128, C1], BF16)
        nc.gpsimd.dma_start(out=comb_sb[:], in_=comb_d[:])
        dsel_sb = consts.tile([128, 1], BF16)
        nc.gpsimd.dma_start(out=dsel_sb[:], in_=dsel_d[:])

        ones_1xC = consts.tile([1, C], BF16)
        nc.vector.memset(ones_1xC[:], 1.0)

        lsum = consts.tile([C, 2], F32)

        UNITS = [[0]] + [list(range(s, s + 3)) for s in range(1, NT, 3)]

        def emit_units(ic, psB, units):
            isl = slice(ic * 512, (ic + 1) * 512)
            for ts_list in units:
                    w = len(ts_list)
                    unit = pln.tile([128, 512 * w], F32, tag="unit")
                    for pos, t in enumerate(ts_list):
                        g = t % 4
                        tloc = t // 4
                        nc.tensor.matmul(
                            unit[:, pos * 512 : (pos + 1) * 512],
                            lhsT=stat_sb[
                                32 * g : 32 * g + K1, tloc * 128 : (tloc + 1) * 128
                            ],
                            rhs=mov_sb[32 * g : 32 * g + K1, isl],
                            start=True,
                            stop=True,
                            tile_position=(32 * g, 0),
                            skip_group_check=True,
                        )
                    for pos, t in enumerate(ts_list):
                        if 4 * ic <= t < 4 * ic + 4:
                            off = pos * 512 + t * 128 - ic * 512
                            nc.vector.tensor_add(
                                unit[:, off : off + 128],
                                unit[:, off : off + 128],
                                diag_sb[:],
                            )
                    ksb = kpool.tile([128, 512 * w], BF16, tag="ksb")
                    nc.scalar.activation(
                        ksb[:], unit[:], mybir.ActivationFunctionType.Exp
                    )
                    for pos, t in enumerate(ts_list):
                        g = t % 4
                        nc.tensor.matmul(
                            psB[32 * g : 32 * g + C1, :],
                            lhsT=y1_sb[:, t, :],
                            rhs=ksb[:, pos * 512 : (pos + 1) * 512],
                            start=(t < 4),
                            stop=(t >= NT - 4),
                            skip_group_check=True,
                            tile_position=(0, 32 * g),
                        )

        def emit_epilogue(ic, psB):
            isl = slice(ic * 512, (ic + 1) * 512)
            # epilogue for this i-chunk (all small tiles)

            ky4 = epool.tile([128, 512], BF16, tag="ky4")
            nc.vector.tensor_copy(ky4[0:116, :], psB[0:116, :])
            psc = pacc.tile([128, 512], F32, tag="psB")
            nc.tensor.matmul(
                psc[32:33, :],
                lhsT=dsel_sb[0:116, :],
                rhs=ky4[0:116, :],
                start=True,
                stop=True,
                skip_group_check=True,
                tile_position=(0, 32),
            )
            nc.tensor.matmul(
                psc[0:C1, :],
                lhsT=comb_sb[0:116, :],
                rhs=ky4[0:116, :],
                start=True,
                stop=True,
                skip_group_check=True,
                tile_position=(0, 0),
            )
            ky = epool.tile([C1, 512], F32, tag="ky")
            if ic == 0:
                nc.vector.tensor_copy(ky[:], psc[0:C1, :])
            else:
                nc.scalar.copy(ky[:], psc[0:C1, :])
            dmx = epool.tile([1, 512], F32, tag="dmx")
            nc.vector.tensor_scalar_max(dmx[:], psc[32:33, :], 1e-10)
            rcp = epool.tile([1, 512], F32, tag="rcp")
            nc.vector.reciprocal_approx_fast(out=rcp[:], in_=dmx[:])
            rcpb = epool.tile([1, 512], BF16, tag="rcpb")
            nc.vector.tensor_copy(rcpb[:], rcp[:])
            psr = pacc.tile([128, 512], F32, tag="psB")
            nc.tensor.matmul(
                psr[0:C, :],
                lhsT=ones_1xC[:],
                rhs=rcpb[:],
                start=True,
                stop=True,
            )
            ratio = epool.tile([C, 512], F32, tag="ratio")
            nc.vector.tensor_mul(ratio[:], ky[0:C, :], psr[0:C, :])
            dd = epool.tile([C, 512], F32, tag="dd")
            nc.vector.tensor_sub(dd[:], ratio[:], ftc_sb[:, isl])
            dd2 = epool.tile([C, 512], F32, tag="dd2")
            nc.vector.tensor_mul(dd2[:], dd[:], dd[:])
            nc.vector.reduce_sum(
                lsum[:, ic : ic + 1], dd2[:], axis=mybir.AxisListType.X
            )

        psB0 = pacc.tile([128, 512], F32, tag="psB")
        emit_units(0, psB0, UNITS)
        psB1 = pacc.tile([128, 512], F32, tag="psB")
        emit_units(1, psB1, UNITS[:7])
        emit_epilogue(0, psB0)
        emit_units(1, psB1, UNITS[7:])
        emit_epilogue(1, psB1)

        lsr = epool.tile([C, 1], F32, tag="lsr")
        nc.vector.reduce_sum(lsr[:], lsum[:], axis=mybir.AxisListType.X)
        part = epool.tile([1, 1], F32, tag="part")
        nc.gpsimd.tensor_reduce(
            part[:], lsr[:], axis=mybir.AxisListType.XYZWC, op=mybir.AluOpType.add
        )
        nc.sync.dma_start(out=out_d[:], in_=part[:])

    nc.compile()
    return nc


_NC_CACHE = None


def _get_nc():
    global _NC_CACHE
    if _NC_CACHE is None:
        _NC_CACHE = _build_nc()
    return _NC_CACHE


def prepare_in_maps(input, target):
    """Host-side preprocessing: softmax/argmax on the strided subsample,
    log-Beta normalizers, bf16 hi/lo operand splits, and the per-core
    j-rotated layouts."""
    x = np.asarray(input)[:, :, ::DF, ::DF].astype(np.float32)
    t = np.asarray(target)[:, :, ::DF, ::DF]

    m = x.max(axis=1, keepdims=True)
    e = np.exp(x - m)
    probs = e / e.sum(axis=1, keepdims=True)
    f = probs.transpose(0, 2, 3, 1).reshape(-1, C).astype(np.float32)
    y = t.argmax(axis=1).reshape(-1)

    alphas = f / BW + np.float32(1.0)
    b = alphas - np.float32(1.0)
    logf = np.log(f)
    lb = (
        gammaln(alphas.astype(np.float64)).sum(axis=1)
        - gammaln(alphas.sum(axis=1, dtype=np.float64))
    ).astype(np.float32)
    nlb = -lb

    in_maps = []
    for k in range(NCORES):
        perm = (np.arange(N) + k * R) % N
        b_rot = b[perm]
        nlb_rot = nlb[perm]
        # j-tile t lives in row-group t%4 (partitions 32g..32g+19), packed
        # at column block t//4 -- no duplication needed for 4-wide matmuls
        stat = np.zeros((128, N // 4), dtype=np.float32)
        for g in range(4):
            sel = np.arange(NT // 4) * 4 + g          # tiles in this group
            cols = (sel[:, None] * 128 + np.arange(128)[None, :]).ravel()
            stat[32 * g : 32 * g + 19] = b_rot[cols].T
            stat[32 * g + 19] = nlb_rot[cols]

        rows = slice(k * R, (k + 1) * R)
        mov = np.zeros((128, R), dtype=np.float32)
        for g in range(4):
            mov[32 * g : 32 * g + 19] = logf[rows].T
            mov[32 * g + 19] = 1.0

        yp = y[perm].reshape(NT, 128)  # [t, p]
        y1 = np.zeros((128, NT, C1), dtype=NPBF16)
        onehot = (yp[:, :, None] == np.arange(C)[None, None, :]).astype(NPBF16)
        y1[:, :, 0:C] = onehot.transpose(1, 0, 2)
        y1[:, :, C] = NPBF16(1.0)

        ftc = np.ascontiguousarray(f[rows].T)

        in_maps.append({"stat": stat, "mov": mov, "y1": y1, "ftc": ftc})
    return in_maps


def run_device(in_maps, trace=False, trace_cores=None):
    nc = _get_nc()
    return run_bass_kernel_spmd(
        nc,
        in_maps,
        core_ids=list(range(NCORES)),
        trace=trace,
        trace_cores=trace_cores,
    )


def kernel(input, target):
    in_maps = prepare_in_maps(input, target)
    res = run_device(in_maps)
    total = np.float32(0.0)
    for r in res.results:
        total += r["out"][0, 0]
    return np.array(total / np.float32(N), dtype=np.float32)

